# revision 35
# baseline (speedup 1.0000x reference)
"""MoE layer (32 experts, top-2, cap=320) on 8 Trainium2 NeuronCores.

Strategy (expert parallel, per sharding hint):
  - Router + dispatch (gating matmul, softmax, top-k, sort, position calc,
    capacity scatter) run on host exactly mirroring the reference's jax ops
    (CPU backend) so routing decisions are bit-identical.
  - The grouped expert FFN - 99.7% of the FLOPs: for each expert,
    gelu(x @ W1 + b1) @ W2 + b2 over [cap=320, 1024] tokens - runs on the 8
    NeuronCores, 4 experts per core, via a Bass/Tile kernel.
  - Everything is computed transposed ([d, tokens] layouts) so the PE array
    needs no on-device transposes: fc1 computes H^T = W1^T-stationary @ X^T,
    which is exactly the moving-operand layout fc2 needs.
  - Weights are reordered host-side into [out_tile][partition, k*128+c]
    "column block" layout so every DMA is 128 partitions x >=2KB contiguous,
    and both W1/W2 stream through SBUF just-in-time (no full residency).
  - Combine (gather, scatter-add, normalize) runs on host jax (CPU).
"""

import os

import numpy as np

P = 128
D_MODEL = 1024
D_FF = 4096
N_EXPERTS = 32
TOPK = 2
CAP_FACTOR = 1.25
B, T = 4, 2048
S = B * T
CAP = 320  # ceil(S / E * 1.25)
N_CORES = 8
ELOC = N_EXPERTS // N_CORES  # 4 experts per core
KD = D_MODEL // P  # 8   fc1 contraction tiles
MF = D_FF // P  # 32  fc1 output tiles == fc2 contraction tiles
MD = D_MODEL // P  # 8   fc2 output tiles

# matmul dtype: "bf16" (compute-bound, ~1e-3 rel err) or "f32r"
# (fp32 storage, TF32-ish matmul, DMA-bound but more accurate)
MM_DTYPE = os.environ.get("KERNEL_MM_DTYPE", "bf16")

_CACHE = {}


# ----------------------------------------------------------------------------
# Host side: router + dispatch + combine (mirrors reference.py bit-exactly)
# ----------------------------------------------------------------------------


def _cpu_device():
    import jax

    return jax.local_devices(backend="cpu")[0]


def _route_and_dispatch(x, w_gate):
    import jax
    import jax.numpy as jnp

    with jax.default_device(_cpu_device()):
        x = jnp.asarray(x)
        w_gate = jnp.asarray(w_gate)
        tokens = x.reshape(-1, D_MODEL)

        logits = tokens @ w_gate.T
        probs = jax.nn.softmax(logits, axis=-1)
        topk_vals, topk_idx = jax.lax.top_k(probs, TOPK)

        importance = probs.sum(axis=0)
        onehot = (
            jnp.zeros((S, N_EXPERTS), jnp.float32)
            .at[jnp.arange(S)[:, None], topk_idx]
            .set(1.0)
        )
        load = onehot.sum(axis=0)
        lb_loss = jnp.mean(importance * load) * (N_EXPERTS**2) / (S * S)

        expert_idx = topk_idx.reshape(-1)
        gate_vals = topk_vals.reshape(-1)
        token_idx = jnp.repeat(jnp.arange(S), TOPK)

        order = jnp.argsort(expert_idx)
        e = expert_idx[order]
        t = token_idx[order]
        g = gate_vals[order]

        one_hot = jax.nn.one_hot(e, N_EXPERTS, dtype=jnp.float32)
        positions = jnp.argmax(jnp.cumsum(one_hot, axis=0), axis=1) - 1
        positions = jnp.minimum(positions, CAP - 1)
        positions = jnp.where(positions < 0, positions + CAP, positions)

        expert_input = (
            jnp.zeros((N_EXPERTS, CAP, D_MODEL), tokens.dtype)
            .at[e, positions]
            .set(tokens[t])
        )

    return (
        np.asarray(expert_input),
        np.asarray(e),
        np.asarray(t),
        np.asarray(g),
        np.asarray(positions),
        np.asarray(lb_loss),
    )


def _combine(expert_output, e, t, g, positions):
    import jax
    import jax.numpy as jnp

    with jax.default_device(_cpu_device()):
        expert_output = jnp.asarray(expert_output)
        e = jnp.asarray(e)
        t = jnp.asarray(t)
        g = jnp.asarray(g)
        positions = jnp.asarray(positions)
        gathered = expert_output[e, positions] * g[:, None]
        combined = jnp.zeros((S, D_MODEL), expert_output.dtype).at[t].add(gathered)
        wsum = jnp.zeros((S,), expert_output.dtype).at[t].add(g)
        denom = jnp.where(wsum > 0, wsum, 1.0)[:, None]
        combined = jnp.where(wsum[:, None] > 0, combined / denom, combined)
        y = combined.reshape(B, T, D_MODEL)
    return np.asarray(y)


# ----------------------------------------------------------------------------
# Device side: Bass/Tile grouped-FFN kernel (ELOC experts per core)
# ----------------------------------------------------------------------------


def _build_ffn_program(
    mm_dtype_name, eloc=ELOC, kd=KD, mf=MF, md=MD, cap=CAP, act="Gelu"
):
    from contextlib import ExitStack

    import concourse.bacc as bacc
    import concourse.mybir as mybir
    import concourse.tile as tile

    ELOC, KD, MF, MD, CAP = eloc, kd, mf, md, cap
    f32 = mybir.dt.float32
    mdt = {"bf16": mybir.dt.bfloat16, "f32r": mybir.dt.float32r}[mm_dtype_name]

    nc = bacc.Bacc("TRN2", target_bir_lowering=False, debug=False)
    W1PAIR = 2  # fc1 weight-column tiles fetched per DMA
    MFP = MF // W1PAIR
    xt = nc.declare_dram_parameter("xt", [ELOC, P, KD * CAP], mdt, isOutput=False)
    w1 = nc.declare_dram_parameter(
        "w1", [ELOC * MFP, P, W1PAIR * KD * P], mdt, isOutput=False
    )
    w2 = nc.declare_dram_parameter("w2", [ELOC * MD, P, MF * P], mdt, isOutput=False)
    b1 = nc.declare_dram_parameter("b1", [P, ELOC * MF], f32, isOutput=False)
    b2 = nc.declare_dram_parameter("b2", [P, ELOC * MD], f32, isOutput=False)
    yt = nc.declare_dram_parameter("yt", [ELOC, P, MD * CAP], f32, isOutput=True)

    gelu = getattr(mybir.ActivationFunctionType, act)

    from concourse.tile import add_dep_helper

    # Each engine's HWDGE queue sustains only ~122GB/s, so the two weight
    # streams (each needing ~216GB/s during its phase) are striped
    # round-robin across all three DMA-capable engines in consumption order.
    # Each DMA is paced to start ~K_AHEAD matmuls before its first consumer
    # so far-future prefetch doesn't starve urgent transfers.
    mm_list = []

    def pace(dma_bi, consumer_idx, lead=64):
        idx = consumer_idx - lead
        if idx >= 0:
            add_dep_helper(dma_bi.ins, mm_list[idx].ins, reason="dma pacing")

    with ExitStack() as ctx:
        tc = ctx.enter_context(tile.TileContext(nc))
        xp = ctx.enter_context(tc.tile_pool(name="xp", bufs=4))
        w1p = ctx.enter_context(tc.tile_pool(name="w1p", bufs=6))
        w2p = ctx.enter_context(tc.tile_pool(name="w2p", bufs=3))
        hp = ctx.enter_context(tc.tile_pool(name="hp", bufs=MF + 8))
        biasp = ctx.enter_context(tc.tile_pool(name="biasp", bufs=4))
        outp = ctx.enter_context(tc.tile_pool(name="outp", bufs=12))
        psp1 = ctx.enter_context(tc.tile_pool(name="psp1", bufs=3, space="PSUM"))
        psp2 = ctx.enter_context(tc.tile_pool(name="psp2", bufs=4, space="PSUM"))

        weng = [nc.sync, nc.gpsimd, nc.scalar]
        wcnt = [0]

        def wdma(dst, src, lead=64):
            eng = weng[wcnt[0] % len(weng)]
            wcnt[0] += 1
            pace(eng.dma_start(dst, src), len(mm_list), lead)

        # PE warm-up: dummy matmuls on zeroed SBUF run while the first real
        # weight/x transfers are in flight, so the HAM clock gate opens
        # (1.2GHz -> 2.4GHz takes ~3.4us of sustained PE activity) before the
        # real stream begins.
        warmpool = ctx.enter_context(tc.tile_pool(name="warmp", bufs=1))
        wpsp = ctx.enter_context(tc.tile_pool(name="wpsp", bufs=1, space="PSUM"))
        junk = warmpool.tile([P, P + CAP], mdt, tag="junk")
        nc.gpsimd.memset(junk[:], 0.0)
        wps = wpsp.tile([P, CAP], f32, tag="warmps")
        for _ in range(26):
            nc.tensor.matmul(
                wps[:], junk[:, :P], junk[:, P : P + CAP], start=True, stop=True
            )

        b1t = biasp.tile([P, ELOC * MF], f32, tag="b1")
        nc.gpsimd.dma_start(b1t[:], b1[:])
        b2t = biasp.tile([P, ELOC * MD], f32, tag="b2")
        nc.gpsimd.dma_start(b2t[:], b2[:])

        KH = KD // 2

        def fc1_unit(ei, mj, xh0, xh1, hts):
            """One w1 pair: DMA + 2 psum groups (16 matmuls) + 2 gelus."""
            w1t = w1p.tile([P, W1PAIR * KD * P], mdt, tag="w1")
            if ei == 0 and mj < 2:
                Q = W1PAIR * KD * P // 4
                for qi in range(4):
                    wdma(
                        w1t[:, qi * Q : (qi + 1) * Q],
                        w1[mj][:, qi * Q : (qi + 1) * Q],
                        lead=96,
                    )
            else:
                wdma(w1t[:], w1[ei * MFP + mj], lead=96)
            for i in range(W1PAIR):
                m = mj * W1PAIR + i
                ps = psp1.tile([P, CAP], f32, tag="ps1")
                for k in range(KD):
                    half = xh0 if k < KH else xh1
                    kk = k % KH
                    mm_list.append(
                        nc.tensor.matmul(
                            ps[:],
                            w1t[:, (i * KD + k) * P : (i * KD + k + 1) * P],
                            half[:, kk * CAP : (kk + 1) * CAP],
                            start=(k == 0),
                            stop=(k == KD - 1),
                        )
                    )
                ht = hp.tile([P, CAP], mdt, tag="h")
                nc.scalar.activation(
                    ht[:], ps[:], gelu, bias=b1t[:, ei * MF + m : ei * MF + m + 1]
                )
                hts.append(ht)

        def fc2_unit(ei, m2, hts):
            """One fc2 output tile: w2 DMA halves + 32-matmul chain + bias+store."""
            w2t = w2p.tile([P, MF * P], mdt, tag="w2")
            HALF = MF * P // 2
            wdma(w2t[:, :HALF], w2[ei * MD + m2][:, :HALF], lead=64)
            wdma(w2t[:, HALF:], w2[ei * MD + m2][:, HALF:], lead=64)
            ps = psp2.tile([P, CAP], f32, tag="ps2")
            for k2 in range(MF):
                mm_list.append(
                    nc.tensor.matmul(
                        ps[:],
                        w2t[:, k2 * P : (k2 + 1) * P],
                        hts[k2][:],
                        start=(k2 == 0),
                        stop=(k2 == MF - 1),
                    )
                )
            ot = outp.tile([P, CAP], f32, tag="out")
            nc.vector.tensor_scalar_add(
                ot[:], ps[:], b2t[:, ei * MD + m2 : ei * MD + m2 + 1]
            )
            nc.scalar.dma_start(yt[ei][:, m2 * CAP : (m2 + 1) * CAP], ot[:])

        # The last 2 fc2 chains of expert e are deferred into the start of
        # expert e+1's fc1 (they are independent), smoothing the weight-DMA
        # phase transition at expert boundaries. Only 2 chains: fc1(e+1)'s
        # h-tiles can take at most the hp pool's spare slots until ALL of
        # expert e's fc2 chains have finished reading h(e).
        DEFER = 2
        pend_fc2 = []
        for ei in range(ELOC):
            # x in two halves so the first psum group isn't gated on the
            # whole 320KB transfer
            xh0 = xp.tile([P, KH * CAP], mdt, tag="x")
            pace(nc.scalar.dma_start(xh0[:], xt[ei][:, : KH * CAP]), len(mm_list))
            xh1 = xp.tile([P, KH * CAP], mdt, tag="x")
            pace(nc.scalar.dma_start(xh1[:], xt[ei][:, KH * CAP :]), len(mm_list))

            hts = []
            fc2_iter = iter(pend_fc2)
            for mj in range(MFP):
                fc1_unit(ei, mj, xh0, xh1, hts)
                if mj in (0, 2):
                    unit = next(fc2_iter, None)
                    if unit is not None:
                        unit()
            pend_fc2 = [
                (lambda e=ei, m=m2, h=hts: fc2_unit(e, m, h)) for m2 in range(MD)
            ]
            for unit in pend_fc2[: MD - DEFER]:
                unit()
            pend_fc2 = pend_fc2[MD - DEFER :]
        for unit in pend_fc2:
            unit()

    nc.compile()
    return nc


def _get_program():
    key = ("prog", MM_DTYPE)
    if key not in _CACHE:
        _CACHE[key] = _build_ffn_program(MM_DTYPE)
    return _CACHE[key]


def _np_mm_dtype():
    if MM_DTYPE == "bf16":
        import ml_dtypes

        return ml_dtypes.bfloat16
    return np.float32


def _ffn_device(expert_input, W1, b1, W2, b2):
    """Run the grouped FFN on the 8 NeuronCores. expert_input: [E, CAP, D] f32.
    Returns expert_output [E, CAP, D] f32."""
    from concourse.bass_utils import run_bass_kernel_spmd

    mdt = _np_mm_dtype()
    W1PAIR = 2
    MFP = MF // W1PAIR

    # per-core input prep (see layout notes in _build_ffn_program)
    # xt: [E, CAP, D] -> [E, KD, P, CAP] -> [E, P, KD*CAP]
    xt_all = (
        expert_input.transpose(0, 2, 1)
        .reshape(N_EXPERTS, KD, P, CAP)
        .transpose(0, 2, 1, 3)
        .reshape(N_EXPERTS, P, KD * CAP)
        .astype(mdt)
    )
    # w1: [E, MFP, P, W1PAIR*KD*P]; [p, (i*KD+k)*P+c] = W1[e, k*P+p, (2mj+i)*P+c]
    w1r = (
        W1.reshape(N_EXPERTS, KD, P, MFP, W1PAIR, P)
        .transpose(0, 3, 2, 4, 1, 5)
        .reshape(N_EXPERTS, MFP, P, W1PAIR * KD * P)
        .astype(mdt)
    )
    w2r = (
        W2.reshape(N_EXPERTS, MF, P, MD, P)
        .transpose(0, 3, 2, 1, 4)
        .reshape(N_EXPERTS, MD, P, MF * P)
        .astype(mdt)
    )
    b1r = b1.reshape(N_EXPERTS, MF, P).astype(np.float32)
    b2r = b2.reshape(N_EXPERTS, MD, P).astype(np.float32)

    in_maps = []
    for c in range(N_CORES):
        sl = slice(c * ELOC, (c + 1) * ELOC)
        in_maps.append(
            {
                "xt": np.ascontiguousarray(xt_all[sl]),
                "w1": np.ascontiguousarray(
                    w1r[sl].reshape(ELOC * MFP, P, W1PAIR * KD * P)
                ),
                "w2": np.ascontiguousarray(w2r[sl].reshape(ELOC * MD, P, MF * P)),
                "b1": np.ascontiguousarray(
                    b1r[sl].transpose(2, 0, 1).reshape(P, ELOC * MF)
                ),
                "b2": np.ascontiguousarray(
                    b2r[sl].transpose(2, 0, 1).reshape(P, ELOC * MD)
                ),
            }
        )

    nc = _get_program()
    kwargs = {}
    if os.environ.get("KERNEL_TRACE"):
        kwargs["trace"] = True
        tmpdir = os.environ.get("KERNEL_TRACE_DIR")
        if tmpdir:
            os.makedirs(tmpdir, exist_ok=True)
            kwargs["tmpdir"] = tmpdir
    res = run_bass_kernel_spmd(nc, in_maps, list(range(N_CORES)), **kwargs)
    _CACHE["last_results"] = res
    outs = res.results

    expert_output = np.empty((N_EXPERTS, CAP, D_MODEL), np.float32)
    for c in range(N_CORES):
        # yt: [ELOC, P, MD*CAP]; [p, m2*CAP+tok] = out[e, tok, m2*P+p]
        ytc = outs[c]["yt"].reshape(ELOC, P, MD, CAP)
        expert_output[c * ELOC : (c + 1) * ELOC] = (
            ytc.transpose(0, 3, 2, 1).reshape(ELOC, CAP, D_MODEL)
        )
    return expert_output


# ----------------------------------------------------------------------------
# Entry point
# ----------------------------------------------------------------------------


def kernel(x, w_gate, W1, b1, W2, b2):
    x = np.asarray(x, dtype=np.float32)
    w_gate = np.asarray(w_gate, dtype=np.float32)
    W1 = np.asarray(W1, dtype=np.float32)
    b1 = np.asarray(b1, dtype=np.float32)
    W2 = np.asarray(W2, dtype=np.float32)
    b2 = np.asarray(b2, dtype=np.float32)

    expert_input, e, t, g, positions, lb_loss = _route_and_dispatch(x, w_gate)
    # Very rarely a device run produces non-finite values (suspected DMA/engine
    # race under power throttling); inputs are deterministic, so retry.
    for _attempt in range(3):
        expert_output = _ffn_device(expert_input, W1, b1, W2, b2)
        if np.isfinite(expert_output).all():
            break
    y = _combine(expert_output, e, t, g, positions)
    return y, lb_loss


# revision 36
# speedup vs baseline: 1.0414x; 1.0414x over previous
"""MoE layer (32 experts, top-2, cap=320) on 8 Trainium2 NeuronCores.

Strategy (expert parallel, per sharding hint):
  - Router + dispatch (gating matmul, softmax, top-k, sort, position calc,
    capacity scatter) run on host exactly mirroring the reference's jax ops
    (CPU backend) so routing decisions are bit-identical.
  - The grouped expert FFN - 99.7% of the FLOPs: for each expert,
    gelu(x @ W1 + b1) @ W2 + b2 over [cap=320, 1024] tokens - runs on the 8
    NeuronCores, 4 experts per core, via a Bass/Tile kernel.
  - Everything is computed transposed ([d, tokens] layouts) so the PE array
    needs no on-device transposes: fc1 computes H^T = W1^T-stationary @ X^T,
    which is exactly the moving-operand layout fc2 needs.
  - Weights are reordered host-side into [out_tile][partition, k*128+c]
    "column block" layout so every DMA is 128 partitions x >=2KB contiguous,
    and both W1/W2 stream through SBUF just-in-time (no full residency).
  - Combine (gather, scatter-add, normalize) runs on host jax (CPU).
"""

import os

import numpy as np

P = 128
D_MODEL = 1024
D_FF = 4096
N_EXPERTS = 32
TOPK = 2
CAP_FACTOR = 1.25
B, T = 4, 2048
S = B * T
CAP = 320  # ceil(S / E * 1.25)
N_CORES = 8
ELOC = N_EXPERTS // N_CORES  # 4 experts per core
KD = D_MODEL // P  # 8   fc1 contraction tiles
MF = D_FF // P  # 32  fc1 output tiles == fc2 contraction tiles
MD = D_MODEL // P  # 8   fc2 output tiles

# matmul dtype: "bf16" (compute-bound, ~1e-3 rel err) or "f32r"
# (fp32 storage, TF32-ish matmul, DMA-bound but more accurate)
MM_DTYPE = os.environ.get("KERNEL_MM_DTYPE", "bf16")

_CACHE = {}


# ----------------------------------------------------------------------------
# Host side: router + dispatch + combine (mirrors reference.py bit-exactly)
# ----------------------------------------------------------------------------


def _cpu_device():
    import jax

    return jax.local_devices(backend="cpu")[0]


def _route_and_dispatch(x, w_gate):
    import jax
    import jax.numpy as jnp

    with jax.default_device(_cpu_device()):
        x = jnp.asarray(x)
        w_gate = jnp.asarray(w_gate)
        tokens = x.reshape(-1, D_MODEL)

        logits = tokens @ w_gate.T
        probs = jax.nn.softmax(logits, axis=-1)
        topk_vals, topk_idx = jax.lax.top_k(probs, TOPK)

        importance = probs.sum(axis=0)
        onehot = (
            jnp.zeros((S, N_EXPERTS), jnp.float32)
            .at[jnp.arange(S)[:, None], topk_idx]
            .set(1.0)
        )
        load = onehot.sum(axis=0)
        lb_loss = jnp.mean(importance * load) * (N_EXPERTS**2) / (S * S)

        expert_idx = topk_idx.reshape(-1)
        gate_vals = topk_vals.reshape(-1)
        token_idx = jnp.repeat(jnp.arange(S), TOPK)

        order = jnp.argsort(expert_idx)
        e = expert_idx[order]
        t = token_idx[order]
        g = gate_vals[order]

        one_hot = jax.nn.one_hot(e, N_EXPERTS, dtype=jnp.float32)
        positions = jnp.argmax(jnp.cumsum(one_hot, axis=0), axis=1) - 1
        positions = jnp.minimum(positions, CAP - 1)
        positions = jnp.where(positions < 0, positions + CAP, positions)

        expert_input = (
            jnp.zeros((N_EXPERTS, CAP, D_MODEL), tokens.dtype)
            .at[e, positions]
            .set(tokens[t])
        )

    return (
        np.asarray(expert_input),
        np.asarray(e),
        np.asarray(t),
        np.asarray(g),
        np.asarray(positions),
        np.asarray(lb_loss),
    )


def _combine(expert_output, e, t, g, positions):
    import jax
    import jax.numpy as jnp

    with jax.default_device(_cpu_device()):
        expert_output = jnp.asarray(expert_output)
        e = jnp.asarray(e)
        t = jnp.asarray(t)
        g = jnp.asarray(g)
        positions = jnp.asarray(positions)
        gathered = expert_output[e, positions] * g[:, None]
        combined = jnp.zeros((S, D_MODEL), expert_output.dtype).at[t].add(gathered)
        wsum = jnp.zeros((S,), expert_output.dtype).at[t].add(g)
        denom = jnp.where(wsum > 0, wsum, 1.0)[:, None]
        combined = jnp.where(wsum[:, None] > 0, combined / denom, combined)
        y = combined.reshape(B, T, D_MODEL)
    return np.asarray(y)


# ----------------------------------------------------------------------------
# Device side: Bass/Tile grouped-FFN kernel (ELOC experts per core)
# ----------------------------------------------------------------------------


def _build_ffn_program(
    mm_dtype_name, eloc=ELOC, kd=KD, mf=MF, md=MD, cap=CAP, act="Gelu"
):
    from contextlib import ExitStack

    import concourse.bacc as bacc
    import concourse.mybir as mybir
    import concourse.tile as tile

    ELOC, KD, MF, MD, CAP = eloc, kd, mf, md, cap
    f32 = mybir.dt.float32
    mdt = {"bf16": mybir.dt.bfloat16, "f32r": mybir.dt.float32r}[mm_dtype_name]

    nc = bacc.Bacc("TRN2", target_bir_lowering=False, debug=False)
    W1PAIR = 2  # fc1 weight-column tiles fetched per DMA
    MFP = MF // W1PAIR
    xt = nc.declare_dram_parameter("xt", [ELOC, P, KD * CAP], mdt, isOutput=False)
    w1 = nc.declare_dram_parameter(
        "w1", [ELOC * MFP, P, W1PAIR * KD * P], mdt, isOutput=False
    )
    w2 = nc.declare_dram_parameter("w2", [ELOC * MD, P, MF * P], mdt, isOutput=False)
    b1 = nc.declare_dram_parameter("b1", [P, ELOC * MF], f32, isOutput=False)
    b2 = nc.declare_dram_parameter("b2", [P, ELOC * MD], f32, isOutput=False)
    yt = nc.declare_dram_parameter("yt", [ELOC, P, MD * CAP], f32, isOutput=True)

    gelu = getattr(mybir.ActivationFunctionType, act)

    from concourse.tile import add_dep_helper

    # Each engine's HWDGE queue sustains only ~122GB/s, so the two weight
    # streams (each needing ~216GB/s during its phase) are striped
    # round-robin across all three DMA-capable engines in consumption order.
    # Each DMA is paced to start ~K_AHEAD matmuls before its first consumer
    # so far-future prefetch doesn't starve urgent transfers.
    mm_list = []

    def pace(dma_bi, consumer_idx, lead=64):
        idx = consumer_idx - lead
        if idx >= 0:
            add_dep_helper(dma_bi.ins, mm_list[idx].ins, reason="dma pacing")

    with ExitStack() as ctx:
        tc = ctx.enter_context(tile.TileContext(nc))
        xp = ctx.enter_context(tc.tile_pool(name="xp", bufs=4))
        w1p = ctx.enter_context(tc.tile_pool(name="w1p", bufs=6))
        w2p = ctx.enter_context(tc.tile_pool(name="w2p", bufs=3))
        hp = ctx.enter_context(tc.tile_pool(name="hp", bufs=MF + 8))
        biasp = ctx.enter_context(tc.tile_pool(name="biasp", bufs=4))
        outp = ctx.enter_context(tc.tile_pool(name="outp", bufs=12))
        psp1 = ctx.enter_context(tc.tile_pool(name="psp1", bufs=3, space="PSUM"))
        psp2 = ctx.enter_context(tc.tile_pool(name="psp2", bufs=4, space="PSUM"))

        weng = [nc.sync, nc.gpsimd, nc.scalar]
        wcnt = [0]

        def wdma(dst, src, lead=64):
            eng = weng[wcnt[0] % len(weng)]
            wcnt[0] += 1
            pace(eng.dma_start(dst, src), len(mm_list), lead)

        # PE warm-up: dummy matmuls on zeroed SBUF run while the first real
        # weight/x transfers are in flight, so the HAM clock gate opens
        # (1.2GHz -> 2.4GHz takes ~3.4us of sustained PE activity) before the
        # real stream begins.
        warmpool = ctx.enter_context(tc.tile_pool(name="warmp", bufs=1))
        wpsp = ctx.enter_context(tc.tile_pool(name="wpsp", bufs=1, space="PSUM"))
        junk = warmpool.tile([P, P + CAP], mdt, tag="junk")
        nc.gpsimd.memset(junk[:], 0.0)
        wps = wpsp.tile([P, CAP], f32, tag="warmps")
        for _ in range(26):
            nc.tensor.matmul(
                wps[:], junk[:, :P], junk[:, P : P + CAP], start=True, stop=True
            )

        b1t = biasp.tile([P, ELOC * MF], f32, tag="b1")
        nc.gpsimd.dma_start(b1t[:], b1[:])
        b2t = biasp.tile([P, ELOC * MD], f32, tag="b2")
        nc.gpsimd.dma_start(b2t[:], b2[:])

        KH = KD // 2

        def fc1_unit(ei, mj, xh0, xh1, hts):
            """One w1 pair: DMA + 2 psum groups (16 matmuls) + 2 gelus."""
            w1t = w1p.tile([P, W1PAIR * KD * P], mdt, tag="w1")
            if mj < 2:
                # first pairs of each expert are the phase-critical arrivals:
                # stripe them across queues (4 parallel quarters) with a
                # deeper lead so the fc1 phase never starts starved
                Q = W1PAIR * KD * P // 4
                for qi in range(4):
                    wdma(
                        w1t[:, qi * Q : (qi + 1) * Q],
                        w1[ei * MFP + mj][:, qi * Q : (qi + 1) * Q],
                        lead=128,
                    )
            else:
                wdma(w1t[:], w1[ei * MFP + mj], lead=96)
            for i in range(W1PAIR):
                m = mj * W1PAIR + i
                ps = psp1.tile([P, CAP], f32, tag="ps1")
                for k in range(KD):
                    half = xh0 if k < KH else xh1
                    kk = k % KH
                    mm_list.append(
                        nc.tensor.matmul(
                            ps[:],
                            w1t[:, (i * KD + k) * P : (i * KD + k + 1) * P],
                            half[:, kk * CAP : (kk + 1) * CAP],
                            start=(k == 0),
                            stop=(k == KD - 1),
                        )
                    )
                ht = hp.tile([P, CAP], mdt, tag="h")
                nc.scalar.activation(
                    ht[:], ps[:], gelu, bias=b1t[:, ei * MF + m : ei * MF + m + 1]
                )
                hts.append(ht)

        def fc2_unit(ei, m2, hts):
            """One fc2 output tile: w2 DMA halves + 32-matmul chain + bias+store."""
            w2t = w2p.tile([P, MF * P], mdt, tag="w2")
            HALF = MF * P // 2
            wdma(w2t[:, :HALF], w2[ei * MD + m2][:, :HALF], lead=64)
            wdma(w2t[:, HALF:], w2[ei * MD + m2][:, HALF:], lead=64)
            ps = psp2.tile([P, CAP], f32, tag="ps2")
            for k2 in range(MF):
                mm_list.append(
                    nc.tensor.matmul(
                        ps[:],
                        w2t[:, k2 * P : (k2 + 1) * P],
                        hts[k2][:],
                        start=(k2 == 0),
                        stop=(k2 == MF - 1),
                    )
                )
            ot = outp.tile([P, CAP], f32, tag="out")
            nc.vector.tensor_scalar_add(
                ot[:], ps[:], b2t[:, ei * MD + m2 : ei * MD + m2 + 1]
            )
            nc.scalar.dma_start(yt[ei][:, m2 * CAP : (m2 + 1) * CAP], ot[:])

        # The last 2 fc2 chains of expert e are deferred into the start of
        # expert e+1's fc1 (they are independent), smoothing the weight-DMA
        # phase transition at expert boundaries. Only 2 chains: fc1(e+1)'s
        # h-tiles can take at most the hp pool's spare slots until ALL of
        # expert e's fc2 chains have finished reading h(e).
        DEFER = 2
        pend_fc2 = []
        for ei in range(ELOC):
            # x in two halves so the first psum group isn't gated on the
            # whole 320KB transfer
            xh0 = xp.tile([P, KH * CAP], mdt, tag="x")
            pace(nc.scalar.dma_start(xh0[:], xt[ei][:, : KH * CAP]), len(mm_list))
            xh1 = xp.tile([P, KH * CAP], mdt, tag="x")
            pace(nc.scalar.dma_start(xh1[:], xt[ei][:, KH * CAP :]), len(mm_list))

            hts = []
            fc2_iter = iter(pend_fc2)
            for mj in range(MFP):
                fc1_unit(ei, mj, xh0, xh1, hts)
                if mj in (0, 2):
                    unit = next(fc2_iter, None)
                    if unit is not None:
                        unit()
            pend_fc2 = [
                (lambda e=ei, m=m2, h=hts: fc2_unit(e, m, h)) for m2 in range(MD)
            ]
            for unit in pend_fc2[: MD - DEFER]:
                unit()
            pend_fc2 = pend_fc2[MD - DEFER :]
        for unit in pend_fc2:
            unit()

    nc.compile()
    return nc


def _get_program():
    key = ("prog", MM_DTYPE)
    if key not in _CACHE:
        _CACHE[key] = _build_ffn_program(MM_DTYPE)
    return _CACHE[key]


def _np_mm_dtype():
    if MM_DTYPE == "bf16":
        import ml_dtypes

        return ml_dtypes.bfloat16
    return np.float32


def _ffn_device(expert_input, W1, b1, W2, b2):
    """Run the grouped FFN on the 8 NeuronCores. expert_input: [E, CAP, D] f32.
    Returns expert_output [E, CAP, D] f32."""
    from concourse.bass_utils import run_bass_kernel_spmd

    mdt = _np_mm_dtype()
    W1PAIR = 2
    MFP = MF // W1PAIR

    # per-core input prep (see layout notes in _build_ffn_program)
    # xt: [E, CAP, D] -> [E, KD, P, CAP] -> [E, P, KD*CAP]
    xt_all = (
        expert_input.transpose(0, 2, 1)
        .reshape(N_EXPERTS, KD, P, CAP)
        .transpose(0, 2, 1, 3)
        .reshape(N_EXPERTS, P, KD * CAP)
        .astype(mdt)
    )
    # w1: [E, MFP, P, W1PAIR*KD*P]; [p, (i*KD+k)*P+c] = W1[e, k*P+p, (2mj+i)*P+c]
    w1r = (
        W1.reshape(N_EXPERTS, KD, P, MFP, W1PAIR, P)
        .transpose(0, 3, 2, 4, 1, 5)
        .reshape(N_EXPERTS, MFP, P, W1PAIR * KD * P)
        .astype(mdt)
    )
    w2r = (
        W2.reshape(N_EXPERTS, MF, P, MD, P)
        .transpose(0, 3, 2, 1, 4)
        .reshape(N_EXPERTS, MD, P, MF * P)
        .astype(mdt)
    )
    b1r = b1.reshape(N_EXPERTS, MF, P).astype(np.float32)
    b2r = b2.reshape(N_EXPERTS, MD, P).astype(np.float32)

    in_maps = []
    for c in range(N_CORES):
        sl = slice(c * ELOC, (c + 1) * ELOC)
        in_maps.append(
            {
                "xt": np.ascontiguousarray(xt_all[sl]),
                "w1": np.ascontiguousarray(
                    w1r[sl].reshape(ELOC * MFP, P, W1PAIR * KD * P)
                ),
                "w2": np.ascontiguousarray(w2r[sl].reshape(ELOC * MD, P, MF * P)),
                "b1": np.ascontiguousarray(
                    b1r[sl].transpose(2, 0, 1).reshape(P, ELOC * MF)
                ),
                "b2": np.ascontiguousarray(
                    b2r[sl].transpose(2, 0, 1).reshape(P, ELOC * MD)
                ),
            }
        )

    nc = _get_program()
    kwargs = {}
    if os.environ.get("KERNEL_TRACE"):
        kwargs["trace"] = True
        tmpdir = os.environ.get("KERNEL_TRACE_DIR")
        if tmpdir:
            os.makedirs(tmpdir, exist_ok=True)
            kwargs["tmpdir"] = tmpdir
    res = run_bass_kernel_spmd(nc, in_maps, list(range(N_CORES)), **kwargs)
    _CACHE["last_results"] = res
    outs = res.results

    expert_output = np.empty((N_EXPERTS, CAP, D_MODEL), np.float32)
    for c in range(N_CORES):
        # yt: [ELOC, P, MD*CAP]; [p, m2*CAP+tok] = out[e, tok, m2*P+p]
        ytc = outs[c]["yt"].reshape(ELOC, P, MD, CAP)
        expert_output[c * ELOC : (c + 1) * ELOC] = (
            ytc.transpose(0, 3, 2, 1).reshape(ELOC, CAP, D_MODEL)
        )
    return expert_output


# ----------------------------------------------------------------------------
# Entry point
# ----------------------------------------------------------------------------


def kernel(x, w_gate, W1, b1, W2, b2):
    x = np.asarray(x, dtype=np.float32)
    w_gate = np.asarray(w_gate, dtype=np.float32)
    W1 = np.asarray(W1, dtype=np.float32)
    b1 = np.asarray(b1, dtype=np.float32)
    W2 = np.asarray(W2, dtype=np.float32)
    b2 = np.asarray(b2, dtype=np.float32)

    expert_input, e, t, g, positions, lb_loss = _route_and_dispatch(x, w_gate)
    # Very rarely a device run produces non-finite values (suspected DMA/engine
    # race under power throttling); inputs are deterministic, so retry.
    for _attempt in range(3):
        expert_output = _ffn_device(expert_input, W1, b1, W2, b2)
        if np.isfinite(expert_output).all():
            break
    y = _combine(expert_output, e, t, g, positions)
    return y, lb_loss
